# revision 42
# baseline (speedup 1.0000x reference)
"""GAU (Gated Attention Unit) layer kernel for Trainium2, 8 NeuronCores.

Sharding: query-sequence-parallel within batch. 4 batches x 2 query slabs
of 2048 -> 8 cores. Each core receives the full 4096-token sequence of its
batch (rows reordered so its own query slab comes first), computes the
full-sequence K/V projection, and attention + output projection for its
own 2048 queries.

v2: all heavy lifting pre-staged on host (h pre-transposed + cast fp8,
Wi/Wo pre-cast fp8 with x16 scale), silu on the ACT engine (the silu
table exists on TRN2 even though CoreSim lacks it), per-partition biases
via the ACT bias path, fp8 DoubleRow output projection, bo folded into
the residual h on host. Per-core dataflow (fp32 PSUM accumulation):
  1a. qk = silu(h@Wi_qk + b) feature-major; gamma/beta + RoPE -> qT,kT
      (qk columns host-permuted evens-first; 1/sqrt(d) folded into
      q_gamma/q_beta host-side)
  1b. v = silu(h@Wi_v + b) token-major [tok,1536] fp8 (bias via ones
      matmul into PSUM; silu in one ACT op per 128-token row)
  1c. u = silu(h@Wi_u + b) feature-major fp8, bias via ACT bias
  2.  two query-pair phases (qph x 1024 tokens): scores^T = kT.T@qT,
      at = relu(s)^2 (ACT relu + DVE square, fp8), Av^T accumulated
      over 32 key tiles fp8-DR, g = u * Av^T fp8, out = g@Wo fp8-DR,
      o = po/65536 + (h+bo), RMS-normalize, DMA out.
"""

import os

import ml_dtypes
import numpy as np

import concourse.bass as bass
import concourse.mybir as mybir
import concourse.tile as tile
from concourse import bacc, bass_utils

P = 128
SEQ = 4096
DIM = 768
NCOL = 3200
UV = 1536
KEY = 128
HALF = 64
SLAB = 2048
KD = DIM // P        # 6 feature k-tiles
KD2 = KD // 2        # 3 DoubleRow feature pairs
KT = SEQ // P        # 32 key-token tiles
KT2 = KT // 2        # 16 DoubleRow key pairs
CH = 512
NCH = SEQ // CH      # 8 token chunks
OWN_CH = SLAB // CH  # 4 own (query) chunks
VT = UV // CH        # 3 v-column chunks
UT = UV // P         # 12 u/v feature tiles
UT2 = UT // 2        # 6 DoubleRow u pairs
NB = 4
NCORES = 8
EPS = 1e-12
QPH = 2              # query-pair phases
QPW = SLAB // QPH    # 1024 tokens per phase
OSC = 1.0 / (16.0 * SEQ)  # output descale: wo x16, at carries xSEQ

F32 = mybir.dt.float32
BF16 = mybir.dt.bfloat16
F8 = mybir.dt.float8e4
OP = mybir.AluOpType
AF = mybir.ActivationFunctionType
DR = mybir.MatmulPerfMode.DoubleRow

_cache = {}
LAST_RESULT = None

# ACT Silu/Gelu tables are broken on this stack (wrong values or exec-unit
# crash); always emit sigmoid + x*sig(x) on DVE.
CFG = {"silu": bool(int(os.environ.get("KSILU", "0")))}


def _build(cfg=None):
    cfg = {**CFG, **(cfg or {})}
    use_silu = cfg["silu"]
    nc = bacc.Bacc(
        "TRN2", target_bir_lowering=False, debug=False, num_devices=NCORES
    )

    def din(name, shape, dt):
        return nc.dram_tensor(name, list(shape), dt, kind="ExternalInput").ap()

    ht8_d = din("ht8", [P, KD, SEQ], F8)     # h pre-transposed, fp8
    wi8v_d = din("wi8v", [P, KD, UV], F8)    # 16*Wi v block
    wi8u_d = din("wi8u", [P, KD, UV], F8)    # 16*Wi u block
    wi8qk_d = din("wi8qk", [P, KD, KEY], F8)  # 16*Wi qk block, permuted
    wo8_d = din("wo8", [P, UT, DIM], F8)     # 16*Wo
    hres_d = din("hres", [SLAB, DIM], F32)   # own-slab h + bo
    # output in bf16 (residual dominates; host casts back to f32)
    cc_d = din("cc", [P, SEQ], BF16)
    ss_d = din("ss", [P, SEQ], BF16)
    gbb_d = din("gbb", [P, 5], F32)          # qg*c, qb*c, kg, kb, b_qk
    bu_d = din("bu", [P, UT], F32)           # bi_u per-partition
    bv8_d = din("bv8", [1, UV], F8)          # 16*bi_v
    out_d = nc.dram_tensor("out", [SLAB, DIM], BF16, kind="ExternalOutput").ap()

    def silu_act(out, in_, bias=0.0, scale=1.0, pool=None, shape=None):
        """silu from PSUM (ACT Silu is broken on this stack): both the
        sigmoid and the x staging run on ACT; DVE does one bf16 mult."""
        if use_silu:
            nc.scalar.activation(
                out=out, in_=in_, func=AF.Silu, bias=bias, scale=scale
            )
        else:
            n = shape[-1] * (shape[1] if len(shape) > 2 else 1)
            sg = pool.tile(list(shape), BF16, tag=f"sg{n}", name="sg", bufs=1)
            nc.scalar.activation(
                out=sg, in_=in_, func=AF.Sigmoid, bias=bias, scale=scale
            )
            xx = pool.tile(list(shape), BF16, tag=f"xx{n}", name="xx", bufs=1)
            if isinstance(bias, float) and bias == 0.0:
                nc.scalar.mul(xx, in_, scale)
            else:
                nc.scalar.activation(
                    out=xx, in_=in_, func=AF.Identity, bias=bias, scale=scale
                )
            nc.vector.tensor_mul(out=out, in0=xx, in1=sg)

    with tile.TileContext(nc) as tc:
        with (
            tc.tile_pool(name="consts", bufs=1) as consts,
            tc.tile_pool(name="persist", bufs=1) as persist,
            tc.tile_pool(name="work", bufs=2) as work,
        ):
            gbb_sb = consts.tile([P, 5], F32, tag="gbb", name="gbb_sb")
            bu_sb = consts.tile([P, UT], F32, tag="bu", name="bu_sb")
            bv8_sb = consts.tile([1, UV], F8, tag="bv8", name="bv8_sb")
            ones8_sb = consts.tile([1, P], F8, tag="ones8", name="ones8_sb")
            eps_sb = consts.tile([P, 1], F32, tag="eps", name="eps_sb")
            nc.sync.dma_start(out=gbb_sb, in_=gbb_d)
            nc.sync.dma_start(out=bu_sb, in_=bu_d)
            nc.sync.dma_start(out=bv8_sb, in_=bv8_d)
            nc.vector.memset(ones8_sb, 1.0)
            nc.vector.memset(eps_sb, EPS)

            p1_cm = tc.tile_pool(name="p1", bufs=1)
            p1 = p1_cm.__enter__()
            ht8 = p1.tile([P, KD, SEQ], F8, tag="ht8", name="ht8")
            wi8v = p1.tile([P, KD, UV], F8, tag="wi8v", name="wi8v")
            wi8u = p1.tile([P, KD, UV], F8, tag="wi8u", name="wi8u")
            wi8qk = p1.tile([P, KD, KEY], F8, tag="wi8qk", name="wi8qk")
            cc_sb = p1.tile([P, SEQ], BF16, tag="cc", name="cc_sb")
            ss_sb = p1.tile([P, SEQ], BF16, tag="ss", name="ss_sb")
            # DMA order matters: v's operands first, split by kd-pair so
            # the first accumulation starts after ~1MB instead of ~5.5MB
            for kd2 in range(KD2):
                sl = slice(2 * kd2, 2 * kd2 + 2)
                nc.sync.dma_start(out=ht8[:, sl, :], in_=ht8_d[:, sl, :])
                nc.sync.dma_start(out=wi8v[:, sl, :], in_=wi8v_d[:, sl, :])
            nc.sync.dma_start(out=wi8qk, in_=wi8qk_d)
            nc.sync.dma_start(out=cc_sb, in_=cc_d)
            nc.sync.dma_start(out=ss_sb, in_=ss_d)
            nc.sync.dma_start(out=wi8u, in_=wi8u_d)

            v8 = persist.tile([P, KT, UV], F8, tag="v8", name="v8")
            kT_sb = persist.tile([P, SEQ], BF16, tag="kT", name="kT_sb")
            qT_sb = persist.tile([P, SLAB], BF16, tag="qT", name="qT_sb")
            u8 = persist.tile([P, UT, SLAB], F8, tag="u8", name="u8")
            wo8 = persist.tile([P, UT, DIM], F8, tag="wo8", name="wo8")
            # chunk-0 scores live here so they can interleave into the u
            # loop (the phase-2 pool opens only after p1 frees)
            at8_0 = persist.tile([P, KT, CH], F8, tag="at80", name="at8_0")
            nc.sync.dma_start(out=wo8, in_=wo8_d)

            # ---- 1a+1b: v (token-major, full seq) with qk chunks
            # interleaved every 8 token-tiles so the rope DVE work
            # overlaps v's PE-heavy phase and kT/qT finish early ----
            def emit_qk(pqp, chp):
                pq = pqp.tile([P, 2, CH], F32, tag="pq", name="pq")
                for kd2 in range(KD2):
                    for chl in range(2):
                        nc.tensor.matmul(
                            pq[:, chl, :],
                            wi8qk[:, 2 * kd2:2 * kd2 + 2, :],
                            ht8[:, 2 * kd2:2 * kd2 + 2,
                                (2 * chp + chl) * CH:(2 * chp + chl + 1) * CH],
                            start=(kd2 == 0), stop=(kd2 == KD2 - 1),
                            perf_mode=DR,
                        )
                for chl in range(2):
                        ch = 2 * chp + chl
                        t0 = ch * CH
                        qk_f = work.tile([P, CH], BF16, tag="qkf", name="qk_f",
                                         bufs=2)
                        silu_act(qk_f, pq[:, chl, :], bias=gbb_sb[:, 4:5],
                                 scale=1.0 / 16, pool=work, shape=[P, CH])
                        targets = [(kT_sb[:, t0:t0 + CH], 2)]
                        if ch < OWN_CH:
                            targets.append((qT_sb[:, t0:t0 + CH], 0))
                        for dst, gi in targets:
                            pre = work.tile([P, CH], BF16, tag="pre",
                                            name="pre", bufs=2)
                            nc.vector.tensor_scalar(
                                out=pre, in0=qk_f,
                                scalar1=gbb_sb[:, gi:gi + 1],
                                scalar2=gbb_sb[:, gi + 1:gi + 2],
                                op0=OP.mult, op1=OP.add,
                            )
                            x1 = pre[0:HALF, :]
                            x2 = pre[HALF:P, :]
                            ta = work.tile([HALF, CH], BF16, tag="ta",
                                           name="ta", bufs=2)
                            tb = work.tile([HALF, CH], BF16, tag="tb",
                                           name="tb", bufs=2)
                            nc.vector.tensor_mul(
                                out=ta, in0=x1, in1=cc_sb[0:HALF, t0:t0 + CH]
                            )
                            nc.vector.tensor_mul(
                                out=tb, in0=x2, in1=ss_sb[HALF:P, t0:t0 + CH]
                            )
                            nc.vector.tensor_sub(
                                out=dst[0:HALF, :], in0=ta, in1=tb
                            )
                            tc_ = work.tile([HALF, CH], BF16, tag="ta",
                                            name="tc_", bufs=2)
                            td = work.tile([HALF, CH], BF16, tag="tb",
                                           name="td", bufs=2)
                            nc.vector.tensor_mul(
                                out=tc_, in0=x1, in1=ss_sb[0:HALF, t0:t0 + CH]
                            )
                            nc.vector.tensor_mul(
                                out=td, in0=x2, in1=cc_sb[HALF:P, t0:t0 + CH]
                            )
                            nc.vector.tensor_add(
                                out=dst[HALF:P, :], in0=tc_, in1=td
                            )

            with (
                tc.tile_pool(name="pv", bufs=2, space="PSUM") as pvp,
                tc.tile_pool(name="pq", bufs=1, space="PSUM") as pqp,
            ):
                for tt in range(KT):
                    pv = pvp.tile([P, VT, CH], F32, tag="pv", name="pv")
                    for vc in range(VT):
                        nc.tensor.matmul(
                            pv[:, vc, :], ones8_sb,
                            bv8_sb[:, vc * CH:(vc + 1) * CH],
                            start=True, stop=False,
                        )
                    for kd2 in range(KD2):
                        for vc in range(VT):
                            nc.tensor.matmul(
                                pv[:, vc, :],
                                ht8[:, 2 * kd2:2 * kd2 + 2,
                                    tt * P:(tt + 1) * P],
                                wi8v[:, 2 * kd2:2 * kd2 + 2,
                                     vc * CH:(vc + 1) * CH],
                                start=False, stop=(kd2 == KD2 - 1),
                                perf_mode=DR,
                            )
                    silu_act(v8[:, tt, 0:2 * CH], pv[:, 0:2, :],
                             scale=1.0 / 16, pool=work, shape=[P, 2 * CH])
                    silu_act(v8[:, tt, 2 * CH:UV], pv[:, 2, :],
                             scale=1.0 / 16, pool=work, shape=[P, CH])
                    if tt % 8 == 7:
                        emit_qk(pqp, tt // 8)

            # ---- 1c + 2: u projection with chunk-0 scores interleaved,
            # then attention software-pipelined over four 512-query
            # chunks — scores(ci+1) interleave into Av(ci) so the ACT
            # relu/square chain always runs one chunk ahead of the PE ----
            ps_s_cm = tc.tile_pool(name="ps_s", bufs=2, space="PSUM")
            ps_s = ps_s_cm.__enter__()

            def score_step(ci, kt, at_slot):
                qc0 = ci * CH
                ps = ps_s.tile([P, CH], F32, tag="ps", name="ps")
                nc.tensor.matmul(
                    ps, kT_sb[:, kt * P:(kt + 1) * P],
                    qT_sb[:, qc0:qc0 + CH],
                    start=True, stop=True,
                )
                rl = work.tile([P, CH], BF16, tag="rl", name="rl", bufs=3)
                nc.scalar.activation(out=rl, in_=ps, func=AF.Relu)
                nc.scalar.activation(
                    out=at_slot[:, kt, :], in_=rl, func=AF.Square
                )

            with tc.tile_pool(name="pu", bufs=2, space="PSUM") as pup:
                for ut in range(UT):
                    for qch in range(2):
                        pu = pup.tile([P, 2, CH], F32, tag="pu", name="pu")
                        for kd2 in range(KD2):
                            for qcl in range(2):
                                nc.tensor.matmul(
                                    pu[:, qcl, :],
                                    wi8u[:, 2 * kd2:2 * kd2 + 2,
                                         ut * P:(ut + 1) * P],
                                    ht8[:, 2 * kd2:2 * kd2 + 2,
                                        (2 * qch + qcl) * CH:
                                        (2 * qch + qcl + 1) * CH],
                                    start=(kd2 == 0), stop=(kd2 == KD2 - 1),
                                    perf_mode=DR,
                                )
                        silu_act(
                            u8[:, ut, qch * 2 * CH:(qch + 1) * 2 * CH],
                            pu[:, :, :], bias=bu_sb[:, ut:ut + 1],
                            scale=1.0 / 16, pool=work, shape=[P, 2 * CH],
                        )
                    # chunk-0 scores: 4 per ut over the last 8 ut
                    # (gives the tail of the rope chain time to finish)
                    if ut >= UT - 8:
                        for kt in range(4 * (ut - UT + 8),
                                        4 * (ut - UT + 8) + 4):
                            score_step(0, kt, at8_0)

            p1_cm.__exit__(None, None, None)
            with (
                tc.tile_pool(name="p2", bufs=1) as p2,
                tc.tile_pool(name="ps_av", bufs=2, space="PSUM") as ps_av,
                tc.tile_pool(name="ps_o", bufs=2, space="PSUM") as ps_o,
            ):
                # chunks 1-3 rotate over two at8 slots (chunk 0 used at8_0)
                at8r = p2.tile([P, KT, 2 * CH], F8, tag="at8r", name="at8r")
                g8 = p2.tile([P, UT, 2 * CH], F8, tag="g8", name="g8")

                def at_slot(ci):
                    if ci == 0:
                        return at8_0
                    h0 = ((ci - 1) % 2) * CH
                    return at8r[:, :, h0:h0 + CH]

                def emit_av(ci, interleave_ci=None):
                    src = at_slot(ci)
                    qc0 = ci * CH
                    for ut in range(UT):
                        pav = ps_av.tile([P, CH], F32, tag="pav", name="pav")
                        for kt2 in range(KT2):
                            nc.tensor.matmul(
                                pav,
                                v8[:, 2 * kt2:2 * kt2 + 2,
                                   ut * P:(ut + 1) * P],
                                src[:, 2 * kt2:2 * kt2 + 2, :],
                                start=(kt2 == 0), stop=(kt2 == KT2 - 1),
                                perf_mode=DR,
                            )
                        nc.vector.scalar_tensor_tensor(
                            out=g8[:, ut, (ci % 2) * CH:(ci % 2 + 1) * CH],
                            in0=pav, scalar=1.0,
                            in1=u8[:, ut, qc0:qc0 + CH],
                            op0=OP.mult, op1=OP.mult,
                        )
                        if interleave_ci is not None:
                            for kt in range(3 * ut, min(3 * ut + 3, KT)):
                                score_step(
                                    interleave_ci, kt, at_slot(interleave_ci)
                                )

                def emit_out(ci):
                    for tl in range(CH // P):
                        tok_l = (ci % 2) * CH + tl * P
                        tok_g = ci * CH + tl * P
                        po_a = ps_o.tile([P, CH], F32, tag="poa", name="po_a")
                        po_b = ps_o.tile([P, DIM - CH], F32, tag="pob",
                                         name="po_b")
                        for u2 in range(UT2):
                            g_t = g8[:, 2 * u2:2 * u2 + 2, tok_l:tok_l + P]
                            nc.tensor.matmul(
                                po_a, g_t, wo8[:, 2 * u2:2 * u2 + 2, 0:CH],
                                start=(u2 == 0), stop=(u2 == UT2 - 1),
                                perf_mode=DR,
                            )
                            nc.tensor.matmul(
                                po_b, g_t, wo8[:, 2 * u2:2 * u2 + 2, CH:DIM],
                                start=(u2 == 0), stop=(u2 == UT2 - 1),
                                perf_mode=DR,
                            )
                        hres = work.tile([P, DIM], F32, tag="hres",
                                         name="hres", bufs=2)
                        nc.sync.dma_start(
                            out=hres, in_=hres_d[tok_g:tok_g + P, :]
                        )
                        o_sb = work.tile([P, DIM], BF16, tag="osb",
                                         name="o_sb", bufs=2)
                        nc.vector.scalar_tensor_tensor(
                            out=o_sb[:, 0:CH], in0=po_a, scalar=OSC,
                            in1=hres[:, 0:CH], op0=OP.mult, op1=OP.add,
                        )
                        nc.vector.scalar_tensor_tensor(
                            out=o_sb[:, CH:DIM], in0=po_b, scalar=OSC,
                            in1=hres[:, CH:DIM], op0=OP.mult, op1=OP.add,
                        )
                        o2 = work.tile([P, DIM], F8, tag="o2", name="o2",
                                       bufs=2)
                        ms = work.tile([P, 1], F32, tag="ms", name="ms")
                        nc.scalar.activation(
                            out=o2, in_=o_sb, func=AF.Square, accum_out=ms
                        )
                        sd = work.tile([P, 1], F32, tag="sd", name="sd")
                        nc.scalar.activation(
                            out=sd, in_=ms, func=AF.Sqrt,
                            bias=eps_sb[:, 0:1], scale=1.0 / DIM,
                        )
                        rinv = work.tile([P, 1], F32, tag="rinv", name="rinv")
                        nc.vector.reciprocal(out=rinv, in_=sd)
                        ofin = work.tile([P, DIM], BF16, tag="ofin",
                                         name="ofin", bufs=2)
                        nc.scalar.mul(ofin, o_sb, rinv[:, 0:1])
                        nc.sync.dma_start(
                            out=out_d[tok_g:tok_g + P, :], in_=ofin
                        )

                for ci in range(OWN_CH):
                    emit_av(
                        ci, interleave_ci=ci + 1 if ci + 1 < OWN_CH else None
                    )
                    emit_out(ci)
            ps_s_cm.__exit__(None, None, None)
    nc.compile()
    return nc


def _get_nc(cfg=None):
    key = ("nc", tuple(sorted((cfg or CFG).items())))
    if key not in _cache:
        _cache[key] = _build(cfg)
    return _cache[key]


def _host_prep(hidden_states, Wi, bi, Wo, bo, q_gamma, q_beta, k_gamma, k_beta):
    h = np.ascontiguousarray(np.asarray(hidden_states, dtype=np.float32))
    Wi = np.asarray(Wi, dtype=np.float32)
    bi = np.asarray(bi, dtype=np.float32)
    Wo = np.asarray(Wo, dtype=np.float32)
    bo = np.asarray(bo, dtype=np.float32)

    perm = np.concatenate([np.arange(0, KEY, 2), np.arange(1, KEY, 2)])
    # scale into e4m3 normal range; 1/16 applied after psum
    wi8 = np.ascontiguousarray(
        (16.0 * Wi).reshape(KD, P, NCOL).transpose(1, 0, 2)
    ).astype(ml_dtypes.float8_e4m3)
    wi8u = np.ascontiguousarray(wi8[:, :, :UV])
    wi8v = np.ascontiguousarray(wi8[:, :, UV:2 * UV])
    wi8qk = np.ascontiguousarray(wi8[:, :, 2 * UV:][:, :, perm])
    wo8 = np.ascontiguousarray(
        (16.0 * Wo).reshape(UT, P, DIM).transpose(1, 0, 2)
    ).astype(ml_dtypes.float8_e4m3)

    c = float(KEY ** -0.5)
    gbb = np.stack(
        [
            np.asarray(q_gamma, np.float32)[perm] * c,
            np.asarray(q_beta, np.float32)[perm] * c,
            np.asarray(k_gamma, np.float32)[perm],
            np.asarray(k_beta, np.float32)[perm],
            bi[2 * UV:][perm],
        ],
        axis=1,
    ).astype(np.float32)
    bu = np.ascontiguousarray(
        bi[:UV].reshape(UT, P).T
    ).astype(np.float32)
    bv8 = (16.0 * bi[UV:2 * UV]).reshape(1, UV).astype(ml_dtypes.float8_e4m3)

    omega = 1.0 / (10000.0 ** (np.arange(HALF, dtype=np.float32) / HALF))
    ang = np.arange(SEQ, dtype=np.float32)[:, None] * omega[None, :]
    cos_t = np.cos(ang).T
    sin_t = np.sin(ang).T
    cc_full = np.concatenate([cos_t, cos_t], axis=0).astype(ml_dtypes.bfloat16)
    ss_full = np.concatenate([sin_t, sin_t], axis=0).astype(ml_dtypes.bfloat16)

    shared = {
        "wi8v": wi8v, "wi8u": wi8u, "wi8qk": wi8qk,
        "wo8": wo8, "gbb": gbb, "bu": bu, "bv8": bv8,
    }
    in_maps = []
    for core in range(NCORES):
        b, s = divmod(core, 2)
        order = np.concatenate(
            [
                np.arange(s * SLAB, (s + 1) * SLAB),
                np.arange((1 - s) * SLAB, (2 - s) * SLAB),
            ]
        )
        hb = h[b][order]
        m = dict(shared)
        m["ht8"] = np.ascontiguousarray(
            hb.T.reshape(KD, P, SEQ).transpose(1, 0, 2)
        ).astype(ml_dtypes.float8_e4m3)
        m["hres"] = np.ascontiguousarray(hb[:SLAB] + bo[None, :])
        m["cc"] = np.ascontiguousarray(cc_full[:, order])
        m["ss"] = np.ascontiguousarray(ss_full[:, order])
        in_maps.append(m)
    return in_maps


def kernel(hidden_states, Wi, bi, Wo, bo, q_gamma, q_beta, k_gamma, k_beta):
    global LAST_RESULT
    nc = _get_nc()
    in_maps = _host_prep(
        hidden_states, Wi, bi, Wo, bo, q_gamma, q_beta, k_gamma, k_beta
    )
    res = bass_utils.run_bass_kernel_spmd(
        nc,
        in_maps,
        core_ids=list(range(NCORES)),
        trace=bool(int(os.environ.get("KTRACE", "0"))),
    )
    LAST_RESULT = res
    out = np.empty((NB, SEQ, DIM), dtype=np.float32)
    for core in range(NCORES):
        b, s = divmod(core, 2)
        out[b, s * SLAB:(s + 1) * SLAB] = res.results[core]["out"].astype(
            np.float32
        )
    return out


# revision 47
# speedup vs baseline: 1.1393x; 1.1393x over previous
"""GAU (Gated Attention Unit) layer kernel for Trainium2, 8 NeuronCores.

Sharding: query-sequence-parallel within batch. 4 batches x 2 query slabs
of 2048 -> 8 cores. Each core receives the full 4096-token sequence of its
batch (rows reordered so its own query slab comes first), computes the
full-sequence K/V projection, and attention + output projection for its
own 2048 queries.

v2: all heavy lifting pre-staged on host (h pre-transposed + cast fp8,
Wi/Wo pre-cast fp8 with x16 scale), silu on the ACT engine (the silu
table exists on TRN2 even though CoreSim lacks it), per-partition biases
via the ACT bias path, fp8 DoubleRow output projection, bo folded into
the residual h on host. Per-core dataflow (fp32 PSUM accumulation):
  1a. qk = silu(h@Wi_qk + b) feature-major; gamma/beta + RoPE -> qT,kT
      (qk columns host-permuted evens-first; 1/sqrt(d) folded into
      q_gamma/q_beta host-side)
  1b. v = silu(h@Wi_v + b) token-major [tok,1536] fp8 (bias via ones
      matmul into PSUM; silu in one ACT op per 128-token row)
  1c. u = silu(h@Wi_u + b) feature-major fp8, bias via ACT bias
  2.  two query-pair phases (qph x 1024 tokens): scores^T = kT.T@qT,
      at = relu(s)^2 (ACT relu + DVE square, fp8), Av^T accumulated
      over 32 key tiles fp8-DR, g = u * Av^T fp8, out = g@Wo fp8-DR,
      o = po/65536 + (h+bo), RMS-normalize, DMA out.
"""

import os

import ml_dtypes
import numpy as np

import concourse.bass as bass
import concourse.mybir as mybir
import concourse.tile as tile
from concourse import bacc, bass_utils

P = 128
SEQ = 4096
DIM = 768
NCOL = 3200
UV = 1536
KEY = 128
HALF = 64
SLAB = 2048
KD = DIM // P        # 6 feature k-tiles
KD2 = KD // 2        # 3 DoubleRow feature pairs
KT = SEQ // P        # 32 key-token tiles
KT2 = KT // 2        # 16 DoubleRow key pairs
CH = 512
NCH = SEQ // CH      # 8 token chunks
OWN_CH = SLAB // CH  # 4 own (query) chunks
VT = UV // CH        # 3 v-column chunks
UT = UV // P         # 12 u/v feature tiles
UT2 = UT // 2        # 6 DoubleRow u pairs
NB = 4
NCORES = 8
EPS = 1e-12
QPH = 2              # query-pair phases
QPW = SLAB // QPH    # 1024 tokens per phase
OSC = 1.0 / (16.0 * SEQ)  # output descale: wo x16, at carries xSEQ

F32 = mybir.dt.float32
BF16 = mybir.dt.bfloat16
F8 = mybir.dt.float8e4
OP = mybir.AluOpType
AF = mybir.ActivationFunctionType
DR = mybir.MatmulPerfMode.DoubleRow

_cache = {}
LAST_RESULT = None

# ACT Silu/Gelu tables are broken on this stack (wrong values or exec-unit
# crash); always emit sigmoid + x*sig(x) on DVE.
CFG = {"silu": bool(int(os.environ.get("KSILU", "0")))}


def _build(cfg=None):
    cfg = {**CFG, **(cfg or {})}
    use_silu = cfg["silu"]
    nc = bacc.Bacc(
        "TRN2", target_bir_lowering=False, debug=False, num_devices=NCORES
    )

    def din(name, shape, dt):
        return nc.dram_tensor(name, list(shape), dt, kind="ExternalInput").ap()

    ht8_d = din("ht8", [P, KD, SEQ], F8)     # h pre-transposed, fp8
    wi8v_d = din("wi8v", [P, KD, UV], F8)    # 16*Wi v block
    wi8u_d = din("wi8u", [P, KD, UV], F8)    # 16*Wi u block
    wi8qk_d = din("wi8qk", [P, KD, KEY], F8)  # 16*Wi qk block, permuted
    wo8_d = din("wo8", [P, UT, DIM], F8)     # 16*Wo
    hres_d = din("hres", [SLAB, DIM], F32)   # own-slab h + bo
    # output in bf16 (residual dominates; host casts back to f32)
    cc_d = din("cc", [P, SEQ], BF16)
    ss_d = din("ss", [P, SEQ], BF16)
    gbb_d = din("gbb", [P, 5], F32)          # qg*c, qb*c, kg, kb, b_qk
    bu_d = din("bu", [P, UT], F32)           # bi_u per-partition
    bv8_d = din("bv8", [1, UV], F8)          # 16*bi_v
    out_d = nc.dram_tensor("out", [SLAB, DIM], BF16, kind="ExternalOutput").ap()

    def silu_act(out, in_, bias=0.0, scale=1.0, pool=None, shape=None):
        """silu from PSUM (ACT Silu is broken on this stack): both the
        sigmoid and the x staging run on ACT; DVE does one bf16 mult."""
        if use_silu:
            nc.scalar.activation(
                out=out, in_=in_, func=AF.Silu, bias=bias, scale=scale
            )
        else:
            n = shape[-1] * (shape[1] if len(shape) > 2 else 1)
            sg = pool.tile(list(shape), BF16, tag=f"sg{n}", name="sg", bufs=1)
            nc.scalar.activation(
                out=sg, in_=in_, func=AF.Sigmoid, bias=bias, scale=scale
            )
            if isinstance(bias, float) and bias == 0.0:
                nc.vector.scalar_tensor_tensor(
                    out=out, in0=in_, scalar=scale, in1=sg,
                    op0=OP.mult, op1=OP.mult,
                )
            else:
                xx = pool.tile(list(shape), BF16, tag=f"xx{n}", name="xx",
                               bufs=1)
                nc.vector.tensor_scalar(
                    out=xx, in0=in_, scalar1=scale, scalar2=bias,
                    op0=OP.mult, op1=OP.add,
                )
                nc.vector.tensor_mul(out=out, in0=xx, in1=sg)

    with tile.TileContext(nc) as tc:
        with (
            tc.tile_pool(name="consts", bufs=1) as consts,
            tc.tile_pool(name="persist", bufs=1) as persist,
            tc.tile_pool(name="work", bufs=2) as work,
        ):
            gbb_sb = consts.tile([P, 5], F32, tag="gbb", name="gbb_sb")
            bu_sb = consts.tile([P, UT], F32, tag="bu", name="bu_sb")
            bv8_sb = consts.tile([1, UV], F8, tag="bv8", name="bv8_sb")
            ones8_sb = consts.tile([1, P], F8, tag="ones8", name="ones8_sb")
            eps_sb = consts.tile([P, 1], F32, tag="eps", name="eps_sb")
            nc.sync.dma_start(out=gbb_sb, in_=gbb_d)
            nc.sync.dma_start(out=bu_sb, in_=bu_d)
            nc.sync.dma_start(out=bv8_sb, in_=bv8_d)
            nc.vector.memset(ones8_sb, 1.0)
            nc.vector.memset(eps_sb, EPS)

            p1_cm = tc.tile_pool(name="p1", bufs=1)
            p1 = p1_cm.__enter__()
            ht8 = p1.tile([P, KD, SEQ], F8, tag="ht8", name="ht8")
            wi8v = p1.tile([P, KD, UV], F8, tag="wi8v", name="wi8v")
            wi8u = p1.tile([P, KD, UV], F8, tag="wi8u", name="wi8u")
            wi8qk = p1.tile([P, KD, KEY], F8, tag="wi8qk", name="wi8qk")
            cc_sb = p1.tile([P, SEQ], BF16, tag="cc", name="cc_sb")
            ss_sb = p1.tile([P, SEQ], BF16, tag="ss", name="ss_sb")
            # DMA order matters: v's operands first, split by kd-pair so
            # the first accumulation starts after ~1MB instead of ~5.5MB
            for kd2 in range(KD2):
                sl = slice(2 * kd2, 2 * kd2 + 2)
                nc.sync.dma_start(out=ht8[:, sl, :], in_=ht8_d[:, sl, :])
                nc.sync.dma_start(out=wi8v[:, sl, :], in_=wi8v_d[:, sl, :])
            nc.sync.dma_start(out=wi8qk, in_=wi8qk_d)
            nc.sync.dma_start(out=cc_sb, in_=cc_d)
            nc.sync.dma_start(out=ss_sb, in_=ss_d)
            nc.sync.dma_start(out=wi8u, in_=wi8u_d)

            v8 = persist.tile([P, KT, UV], F8, tag="v8", name="v8")
            kT_sb = persist.tile([P, SEQ], BF16, tag="kT", name="kT_sb")
            qT_sb = persist.tile([P, SLAB], BF16, tag="qT", name="qT_sb")
            u8 = persist.tile([P, UT, SLAB], F8, tag="u8", name="u8")
            wo8 = persist.tile([P, UT, DIM], F8, tag="wo8", name="wo8")
            # chunk-0 scores live here so they can interleave into the u
            # loop (the phase-2 pool opens only after p1 frees)
            at8_0 = persist.tile([P, KT, CH], F8, tag="at80", name="at8_0")
            nc.sync.dma_start(out=wo8, in_=wo8_d)

            # ---- 1a+1b: v (token-major, full seq) with qk chunks
            # interleaved every 8 token-tiles so the rope DVE work
            # overlaps v's PE-heavy phase and kT/qT finish early ----
            def emit_qk(pqp, chp):
                pq = pqp.tile([P, 2, CH], F32, tag="pq", name="pq")
                for kd2 in range(KD2):
                    for chl in range(2):
                        nc.tensor.matmul(
                            pq[:, chl, :],
                            wi8qk[:, 2 * kd2:2 * kd2 + 2, :],
                            ht8[:, 2 * kd2:2 * kd2 + 2,
                                (2 * chp + chl) * CH:(2 * chp + chl + 1) * CH],
                            start=(kd2 == 0), stop=(kd2 == KD2 - 1),
                            perf_mode=DR,
                        )
                for chl in range(2):
                        ch = 2 * chp + chl
                        t0 = ch * CH
                        qk_f = work.tile([P, CH], BF16, tag="qkf", name="qk_f",
                                         bufs=2)
                        silu_act(qk_f, pq[:, chl, :], bias=gbb_sb[:, 4:5],
                                 scale=1.0 / 16, pool=work, shape=[P, CH])
                        targets = [(kT_sb[:, t0:t0 + CH], 2)]
                        if ch < OWN_CH:
                            targets.append((qT_sb[:, t0:t0 + CH], 0))
                        for dst, gi in targets:
                            pre = work.tile([P, CH], BF16, tag="pre",
                                            name="pre", bufs=1)
                            nc.vector.tensor_scalar(
                                out=pre, in0=qk_f,
                                scalar1=gbb_sb[:, gi:gi + 1],
                                scalar2=gbb_sb[:, gi + 1:gi + 2],
                                op0=OP.mult, op1=OP.add,
                            )
                            x1 = pre[0:HALF, :]
                            x2 = pre[HALF:P, :]
                            ta = work.tile([HALF, CH], BF16, tag="ta",
                                           name="ta", bufs=1)
                            tb = work.tile([HALF, CH], BF16, tag="tb",
                                           name="tb", bufs=1)
                            nc.vector.tensor_mul(
                                out=ta, in0=x1, in1=cc_sb[0:HALF, t0:t0 + CH]
                            )
                            nc.vector.tensor_mul(
                                out=tb, in0=x2, in1=ss_sb[HALF:P, t0:t0 + CH]
                            )
                            nc.vector.tensor_sub(
                                out=dst[0:HALF, :], in0=ta, in1=tb
                            )
                            tc_ = work.tile([HALF, CH], BF16, tag="ta",
                                            name="tc_", bufs=1)
                            td = work.tile([HALF, CH], BF16, tag="tb",
                                           name="td", bufs=1)
                            nc.vector.tensor_mul(
                                out=tc_, in0=x1, in1=ss_sb[0:HALF, t0:t0 + CH]
                            )
                            nc.vector.tensor_mul(
                                out=td, in0=x2, in1=cc_sb[HALF:P, t0:t0 + CH]
                            )
                            nc.vector.tensor_add(
                                out=dst[HALF:P, :], in0=tc_, in1=td
                            )

            with (
                tc.tile_pool(name="pv", bufs=2, space="PSUM") as pvp,
                tc.tile_pool(name="pq", bufs=1, space="PSUM") as pqp,
            ):
                for tt in range(KT):
                    pv = pvp.tile([P, VT, CH], F32, tag="pv", name="pv")
                    for vc in range(VT):
                        nc.tensor.matmul(
                            pv[:, vc, :], ones8_sb,
                            bv8_sb[:, vc * CH:(vc + 1) * CH],
                            start=True, stop=False,
                        )
                    for kd2 in range(KD2):
                        for vc in range(VT):
                            nc.tensor.matmul(
                                pv[:, vc, :],
                                ht8[:, 2 * kd2:2 * kd2 + 2,
                                    tt * P:(tt + 1) * P],
                                wi8v[:, 2 * kd2:2 * kd2 + 2,
                                     vc * CH:(vc + 1) * CH],
                                start=False, stop=(kd2 == KD2 - 1),
                                perf_mode=DR,
                            )
                    silu_act(v8[:, tt, :], pv[:, :, :],
                             scale=1.0 / 16, pool=work, shape=[P, UV])
                    if tt % 8 == 7:
                        emit_qk(pqp, tt // 8)

            # ---- 1c + 2: u projection with chunk-0 scores interleaved,
            # then attention software-pipelined over four 512-query
            # chunks — scores(ci+1) interleave into Av(ci) so the ACT
            # relu/square chain always runs one chunk ahead of the PE ----
            ps_s_cm = tc.tile_pool(name="ps_s", bufs=2, space="PSUM")
            ps_s = ps_s_cm.__enter__()

            def score_step(ci, kt, at_slot):
                qc0 = ci * CH
                ps = ps_s.tile([P, CH], F32, tag="ps", name="ps")
                nc.tensor.matmul(
                    ps, kT_sb[:, kt * P:(kt + 1) * P],
                    qT_sb[:, qc0:qc0 + CH],
                    start=True, stop=True,
                )
                rl = work.tile([P, CH], BF16, tag="rl", name="rl", bufs=2)
                nc.scalar.activation(out=rl, in_=ps, func=AF.Relu)
                nc.vector.tensor_mul(out=at_slot[:, kt, :], in0=rl, in1=rl)

            with tc.tile_pool(name="pu", bufs=2, space="PSUM") as pup:
                for ut in range(UT):
                    for qch in range(2):
                        pu = pup.tile([P, 2, CH], F32, tag="pu", name="pu")
                        for kd2 in range(KD2):
                            for qcl in range(2):
                                nc.tensor.matmul(
                                    pu[:, qcl, :],
                                    wi8u[:, 2 * kd2:2 * kd2 + 2,
                                         ut * P:(ut + 1) * P],
                                    ht8[:, 2 * kd2:2 * kd2 + 2,
                                        (2 * qch + qcl) * CH:
                                        (2 * qch + qcl + 1) * CH],
                                    start=(kd2 == 0), stop=(kd2 == KD2 - 1),
                                    perf_mode=DR,
                                )
                        silu_act(
                            u8[:, ut, qch * 2 * CH:(qch + 1) * 2 * CH],
                            pu[:, :, :], bias=bu_sb[:, ut:ut + 1],
                            scale=1.0 / 16, pool=work, shape=[P, 2 * CH],
                        )
                    # chunk-0 scores: 4 per ut over the last 8 ut
                    # (gives the tail of the rope chain time to finish)
                    if ut >= UT - 8:
                        for kt in range(4 * (ut - UT + 8),
                                        4 * (ut - UT + 8) + 4):
                            score_step(0, kt, at8_0)

            p1_cm.__exit__(None, None, None)
            with (
                tc.tile_pool(name="p2", bufs=1) as p2,
                tc.tile_pool(name="ps_av", bufs=2, space="PSUM") as ps_av,
                tc.tile_pool(name="ps_o", bufs=2, space="PSUM") as ps_o,
            ):
                # chunks 1-3 rotate over two at8 slots (chunk 0 used at8_0)
                at8r = p2.tile([P, KT, 2 * CH], F8, tag="at8r", name="at8r")
                g8 = p2.tile([P, UT, 2 * CH], F8, tag="g8", name="g8")

                def at_slot(ci):
                    if ci == 0:
                        return at8_0
                    h0 = ((ci - 1) % 2) * CH
                    return at8r[:, :, h0:h0 + CH]

                def emit_av(ci, interleave_ci=None):
                    src = at_slot(ci)
                    qc0 = ci * CH
                    for ut in range(UT):
                        pav = ps_av.tile([P, CH], F32, tag="pav", name="pav")
                        for kt2 in range(KT2):
                            nc.tensor.matmul(
                                pav,
                                v8[:, 2 * kt2:2 * kt2 + 2,
                                   ut * P:(ut + 1) * P],
                                src[:, 2 * kt2:2 * kt2 + 2, :],
                                start=(kt2 == 0), stop=(kt2 == KT2 - 1),
                                perf_mode=DR,
                            )
                        nc.vector.scalar_tensor_tensor(
                            out=g8[:, ut, (ci % 2) * CH:(ci % 2 + 1) * CH],
                            in0=pav, scalar=1.0,
                            in1=u8[:, ut, qc0:qc0 + CH],
                            op0=OP.mult, op1=OP.mult,
                        )
                        if interleave_ci is not None:
                            for kt in range(3 * ut, min(3 * ut + 3, KT)):
                                score_step(
                                    interleave_ci, kt, at_slot(interleave_ci)
                                )

                def emit_out(ci):
                    for tl in range(CH // P):
                        tok_l = (ci % 2) * CH + tl * P
                        tok_g = ci * CH + tl * P
                        po_a = ps_o.tile([P, CH], F32, tag="poa", name="po_a")
                        po_b = ps_o.tile([P, DIM - CH], F32, tag="pob",
                                         name="po_b")
                        for u2 in range(UT2):
                            g_t = g8[:, 2 * u2:2 * u2 + 2, tok_l:tok_l + P]
                            nc.tensor.matmul(
                                po_a, g_t, wo8[:, 2 * u2:2 * u2 + 2, 0:CH],
                                start=(u2 == 0), stop=(u2 == UT2 - 1),
                                perf_mode=DR,
                            )
                            nc.tensor.matmul(
                                po_b, g_t, wo8[:, 2 * u2:2 * u2 + 2, CH:DIM],
                                start=(u2 == 0), stop=(u2 == UT2 - 1),
                                perf_mode=DR,
                            )
                        hres = work.tile([P, DIM], F32, tag="hres",
                                         name="hres", bufs=2)
                        nc.sync.dma_start(
                            out=hres, in_=hres_d[tok_g:tok_g + P, :]
                        )
                        o_sb = work.tile([P, DIM], BF16, tag="osb",
                                         name="o_sb", bufs=2)
                        nc.vector.scalar_tensor_tensor(
                            out=o_sb[:, 0:CH], in0=po_a, scalar=OSC,
                            in1=hres[:, 0:CH], op0=OP.mult, op1=OP.add,
                        )
                        nc.vector.scalar_tensor_tensor(
                            out=o_sb[:, CH:DIM], in0=po_b, scalar=OSC,
                            in1=hres[:, CH:DIM], op0=OP.mult, op1=OP.add,
                        )
                        o2 = work.tile([P, DIM], F8, tag="o2", name="o2",
                                       bufs=2)
                        ms = work.tile([P, 1], F32, tag="ms", name="ms")
                        nc.scalar.activation(
                            out=o2, in_=o_sb, func=AF.Square, accum_out=ms
                        )
                        sd = work.tile([P, 1], F32, tag="sd", name="sd")
                        nc.scalar.activation(
                            out=sd, in_=ms, func=AF.Sqrt,
                            bias=eps_sb[:, 0:1], scale=1.0 / DIM,
                        )
                        rinv = work.tile([P, 1], F32, tag="rinv", name="rinv")
                        nc.vector.reciprocal(out=rinv, in_=sd)
                        ofin = work.tile([P, DIM], BF16, tag="ofin",
                                         name="ofin", bufs=2)
                        nc.scalar.mul(ofin, o_sb, rinv[:, 0:1])
                        nc.sync.dma_start(
                            out=out_d[tok_g:tok_g + P, :], in_=ofin
                        )

                for ci in range(OWN_CH):
                    emit_av(
                        ci, interleave_ci=ci + 1 if ci + 1 < OWN_CH else None
                    )
                    emit_out(ci)
            ps_s_cm.__exit__(None, None, None)
    nc.compile()
    return nc


def _get_nc(cfg=None):
    key = ("nc", tuple(sorted((cfg or CFG).items())))
    if key not in _cache:
        _cache[key] = _build(cfg)
    return _cache[key]


def _host_prep(hidden_states, Wi, bi, Wo, bo, q_gamma, q_beta, k_gamma, k_beta):
    h = np.ascontiguousarray(np.asarray(hidden_states, dtype=np.float32))
    Wi = np.asarray(Wi, dtype=np.float32)
    bi = np.asarray(bi, dtype=np.float32)
    Wo = np.asarray(Wo, dtype=np.float32)
    bo = np.asarray(bo, dtype=np.float32)

    perm = np.concatenate([np.arange(0, KEY, 2), np.arange(1, KEY, 2)])
    # scale into e4m3 normal range; 1/16 applied after psum
    wi8 = np.ascontiguousarray(
        (16.0 * Wi).reshape(KD, P, NCOL).transpose(1, 0, 2)
    ).astype(ml_dtypes.float8_e4m3)
    wi8u = np.ascontiguousarray(wi8[:, :, :UV])
    wi8v = np.ascontiguousarray(wi8[:, :, UV:2 * UV])
    wi8qk = np.ascontiguousarray(wi8[:, :, 2 * UV:][:, :, perm])
    wo8 = np.ascontiguousarray(
        (16.0 * Wo).reshape(UT, P, DIM).transpose(1, 0, 2)
    ).astype(ml_dtypes.float8_e4m3)

    c = float(KEY ** -0.5)
    gbb = np.stack(
        [
            np.asarray(q_gamma, np.float32)[perm] * c,
            np.asarray(q_beta, np.float32)[perm] * c,
            np.asarray(k_gamma, np.float32)[perm],
            np.asarray(k_beta, np.float32)[perm],
            bi[2 * UV:][perm],
        ],
        axis=1,
    ).astype(np.float32)
    bu = np.ascontiguousarray(
        bi[:UV].reshape(UT, P).T
    ).astype(np.float32)
    bv8 = (16.0 * bi[UV:2 * UV]).reshape(1, UV).astype(ml_dtypes.float8_e4m3)

    omega = 1.0 / (10000.0 ** (np.arange(HALF, dtype=np.float32) / HALF))
    ang = np.arange(SEQ, dtype=np.float32)[:, None] * omega[None, :]
    cos_t = np.cos(ang).T
    sin_t = np.sin(ang).T
    cc_full = np.concatenate([cos_t, cos_t], axis=0).astype(ml_dtypes.bfloat16)
    ss_full = np.concatenate([sin_t, sin_t], axis=0).astype(ml_dtypes.bfloat16)

    shared = {
        "wi8v": wi8v, "wi8u": wi8u, "wi8qk": wi8qk,
        "wo8": wo8, "gbb": gbb, "bu": bu, "bv8": bv8,
    }
    in_maps = []
    for core in range(NCORES):
        b, s = divmod(core, 2)
        order = np.concatenate(
            [
                np.arange(s * SLAB, (s + 1) * SLAB),
                np.arange((1 - s) * SLAB, (2 - s) * SLAB),
            ]
        )
        hb = h[b][order]
        m = dict(shared)
        m["ht8"] = np.ascontiguousarray(
            hb.T.reshape(KD, P, SEQ).transpose(1, 0, 2)
        ).astype(ml_dtypes.float8_e4m3)
        m["hres"] = np.ascontiguousarray(hb[:SLAB] + bo[None, :])
        m["cc"] = np.ascontiguousarray(cc_full[:, order])
        m["ss"] = np.ascontiguousarray(ss_full[:, order])
        in_maps.append(m)
    return in_maps


def kernel(hidden_states, Wi, bi, Wo, bo, q_gamma, q_beta, k_gamma, k_beta):
    global LAST_RESULT
    nc = _get_nc()
    in_maps = _host_prep(
        hidden_states, Wi, bi, Wo, bo, q_gamma, q_beta, k_gamma, k_beta
    )
    res = bass_utils.run_bass_kernel_spmd(
        nc,
        in_maps,
        core_ids=list(range(NCORES)),
        trace=bool(int(os.environ.get("KTRACE", "0"))),
    )
    LAST_RESULT = res
    out = np.empty((NB, SEQ, DIM), dtype=np.float32)
    for core in range(NCORES):
        b, s = divmod(core, 2)
        out[b, s * SLAB:(s + 1) * SLAB] = res.results[core]["out"].astype(
            np.float32
        )
    return out
